# revision 2
# baseline (speedup 1.0000x reference)
"""MultiHeadGAT layer on 8 trn2 NeuronCores, data-parallel over batch — v7.

Math (per core, softmax scale-invariance):
  P'[j,i] = adj[i,j] * max(u[i]*v1[j], v2[j]);  u=exp(.8 a), v1=exp(b),
  v2=exp(.2 b);  a=e_src, b=e_dst.  Exact: the exp(.2 a_i) factor cancels
  in the softmax.

Per [128j, 1024i] tile (pure DVE path; ACT/GPSIMD offloads measured
net-negative: scalar_tensor_tensor runs 1x, GPSIMD contends for the DVE
SBUF port):
  build: fused tensor_scalar  m = (u_b mult v1col) max v2col   (bf16 4x)
  mask:  tensor_tensor paired over 2 heads w/ 0-stride repeat AP on adjT
AV matmul i-on-partitions: lhsT = P-tile [128,128] slices (FWL), rhs =
wh_aug head slice [64 Wh cols | ones] -> denominator in acc col 65k+64.
Epilogue: strided reciprocal + ACT scale-copy into staged bf16 tiles,
half-width output DMAs after hp1/hp3.  Prologue work (whaug, v12) is
interleaved into head-pair 0's jb loop so PE/ACT aren't front-loaded.
Host-side (bf16): hP/adjP pre-transposed+packed, wpack = Waug|WA rows.
"""
import sys

sys.path.insert(0, "/opt/trn_rl_repo")

import numpy as np
import ml_dtypes

import concourse.bass as bass
import concourse.mybir as mybir
import concourse.tile as tile
from concourse.bass_utils import run_bass_kernel_spmd
from concourse.masks import make_identity

F32 = mybir.dt.float32
BF16 = mybir.dt.bfloat16
AF = mybir.ActivationFunctionType
OP = mybir.AluOpType

N_CORES = 8
N = 1024
NB = 8
FIN = 256
H = 8
FOH = 64
FO = H * FOH
AUG = FOH + 1
ALPHA = 0.2
WAUGC = H * AUG  # 520
NSEL = 2         # heads built via PE selector; rest via DMA doubling

_MAX_SYNC_WAITS = 1


def _split_sync_waits(nc, max_waits=_MAX_SYNC_WAITS):
    """This walrus build rejects instructions carrying more than one sync
    wait; hoist extras onto NOPs inserted just before, on the same engine."""
    uid = 0
    for f in nc.m.functions:
        for bb in f.blocks:
            out = []
            for inst in bb.instructions:
                si = getattr(inst, "sync_info", None)
                if si is not None and si.on_wait and len(si.on_wait) > max_waits:
                    waits = list(si.on_wait)
                    keep = waits[-max_waits:]
                    extra = waits[:-max_waits]
                    si.on_wait.clear()
                    si.on_wait.extend(keep)
                    while extra:
                        chunk, extra = extra[:max_waits], extra[max_waits:]
                        nop = mybir.InstNoOp(
                            name=f"waitsplit-{uid}",
                            engine=inst.engine,
                            sync_info=mybir.SyncInfo(
                                on_wait=list(chunk), on_update=[]
                            ),
                            bass_nofuse=True,
                        )
                        uid += 1
                        out.append(nop)
                out.append(inst)
            bb.instructions[:] = out


def build_nc(split=True):
    nc = bass.Bass()
    hp_d = nc.declare_dram_parameter("hP", [128, 2 * N], BF16, isOutput=False)
    adjp_d = nc.declare_dram_parameter("adjP", [128, NB * N], BF16,
                                       isOutput=False)
    wpack_d = nc.declare_dram_parameter("wpack", [128, 2 * WAUGC + 4 * H],
                                        BF16, isOutput=False)
    wones_d = nc.declare_dram_parameter("wones", [1, WAUGC], BF16,
                                        isOutput=False)
    out_d = nc.declare_dram_parameter("out", [N, FO], BF16, isOutput=True)

    with tile.TileContext(nc) as tc:
        with (
            tc.tile_pool(name="const", bufs=1) as const,
            tc.tile_pool(name="persist", bufs=1) as persist,
            tc.tile_pool(name="mp", bufs=4) as mp,
            tc.tile_pool(name="epi", bufs=2) as epi,
            tc.tile_pool(name="psS", bufs=2, space="PSUM") as psS,
            tc.tile_pool(name="psAcc", bufs=1, space="PSUM") as psAcc,
        ):
            # ---- input DMAs first ----
            wpk = const.tile([128, 2 * WAUGC + 4 * H], BF16, tag="wpk")
            nc.sync.dma_start(wpk[:], wpack_d[:, :])
            wa0 = wpk[:, 0:WAUGC]
            wa1 = wpk[:, WAUGC:2 * WAUGC]
            wak0 = wpk[:, 2 * WAUGC:2 * WAUGC + 2 * H]
            wak1 = wpk[:, 2 * WAUGC + 2 * H:2 * WAUGC + 4 * H]
            wones = const.tile([1, WAUGC], BF16, tag="wones")
            nc.sync.dma_start(wones[:], wones_d[:, :])

            htb = const.tile([128, 2 * N], BF16, tag="htb")
            nc.sync.dma_start(htb[:], hp_d[:, :])
            ht0 = htb[:, 0:N]
            ht1 = htb[:, N:2 * N]

            # adjacency: per-block DMAs so each jb is usable on arrival
            adjb = persist.tile([128, NB * N], BF16, tag="adjb")
            for jb in range(NB):
                nc.sync.dma_start(adjb[:, jb * N:(jb + 1) * N],
                                  adjp_d[:, jb * N:(jb + 1) * N])

            def adjt(jb):
                return adjb[:, jb * N:(jb + 1) * N]

            ident = const.tile([128, 128], F32, tag="ident")
            make_identity(nc, ident[:])
            onesrow = const.tile([1, 128], BF16, tag="onesrow")
            nc.gpsimd.memset(onesrow[:], 1.0)

            # preheat the ACT exp table set while DMAs run
            ph = const.tile([1, 16], F32, tag="ph")
            nc.gpsimd.memset(ph[:], 0.0)
            pho = const.tile([1, 16], F32, tag="pho")
            nc.scalar.activation(pho[:], ph[:], AF.Exp)

            # ---- E_T[16, i]: rows 0..7 e_src, 8..15 e_dst ----
            e_t = const.tile([16, N], F32, tag="eT")
            for c in range(2):
                ps = psS.tile([128, 512], F32, tag="ps")
                nc.tensor.matmul(ps[0:16, :], wak0,
                                 ht0[:, c * 512:(c + 1) * 512],
                                 start=True, stop=False)
                nc.tensor.matmul(ps[0:16, :], wak1,
                                 ht1[:, c * 512:(c + 1) * 512],
                                 start=False, stop=True)
                nc.vector.tensor_copy(e_t[:, c * 512:(c + 1) * 512],
                                      ps[0:16, :])

            # ---- u rows; broadcast: first NSEL heads via PE selector
            # (low latency), rest via DMA log-doubling (no engine cost) ----
            urow = const.tile([H, N], BF16, tag="urow")
            nc.scalar.activation(urow[:], e_t[0:H, :], AF.Exp, scale=0.8)
            ub = [persist.tile([128, N], BF16, tag=f"ub{hh}", name=f"ub{hh}")
                  for hh in range(H)]
            for hh in range(NSEL):
                sel = const.tile([H, 128], BF16, tag=f"sel{hh}",
                                 name=f"sel{hh}")
                nc.gpsimd.memset(sel[:], 0.0)
                nc.gpsimd.affine_select(
                    out=sel[:], in_=sel[:], pattern=[[0, 128]],
                    compare_op=mybir.AluOpType.not_equal, fill=1.0,
                    base=-hh, channel_multiplier=1,
                )
                for c in range(2):
                    ps = psS.tile([128, 512], F32, tag="ps")
                    nc.tensor.matmul(ps[:], sel[:],
                                     urow[:, c * 512:(c + 1) * 512],
                                     start=True, stop=True)
                    if c == 0:
                        nc.vector.tensor_copy(
                            ub[hh][:, c * 512:(c + 1) * 512], ps[:])
                    else:
                        nc.scalar.copy(
                            ub[hh][:, c * 512:(c + 1) * 512], ps[:])
            for hh in range(NSEL, H):
                nc.sync.dma_start(ub[hh][0:1, :], urow[hh:hh + 1, :])
                p = 1
                while p < 128:
                    nc.sync.dma_start(ub[hh][p:2 * p, :], ub[hh][0:p, :])
                    p *= 2

            # ---- deferred prologue pieces, emitted inside hp0's jb loop ----
            v12 = [persist.tile([128, 2 * H], F32, tag=f"v{j}", name=f"v{j}")
                   for j in range(NB)]
            whaug = [persist.tile([128, WAUGC], BF16, tag=f"wha{j}",
                                  name=f"wha{j}")
                     for j in range(NB)]

            def emit_v12(jb):
                tp = psS.tile([128, 512], F32, tag="ps")
                nc.tensor.transpose(tp[:, 0:16],
                                    e_t[:, jb * 128:(jb + 1) * 128],
                                    ident[0:16, 0:16])
                nc.scalar.activation(v12[jb][:, 0:H], tp[:, 8:16], AF.Exp)
                nc.scalar.activation(v12[jb][:, H:2 * H], tp[:, 8:16],
                                     AF.Exp, scale=ALPHA)

            def emit_whaug(jb):
                for half in range(2):
                    cs = half * (WAUGC // 2)
                    ce = (half + 1) * (WAUGC // 2)
                    ps = psS.tile([128, 512], F32, tag="ps")
                    nc.tensor.matmul(ps[:, 0:ce - cs],
                                     ht0[:, jb * 128:(jb + 1) * 128],
                                     wa0[:, cs:ce], start=True, stop=False)
                    nc.tensor.matmul(ps[:, 0:ce - cs],
                                     ht1[:, jb * 128:(jb + 1) * 128],
                                     wa1[:, cs:ce], start=False, stop=False)
                    nc.tensor.matmul(ps[:, 0:ce - cs], onesrow[:],
                                     wones[:, cs:ce], start=False, stop=True)
                    if half == 0:
                        nc.vector.tensor_copy(whaug[jb][:, cs:ce],
                                              ps[:, 0:ce - cs])
                    else:
                        nc.scalar.copy(whaug[jb][:, cs:ce], ps[:, 0:ce - cs])

            # ---- output staging ----
            stg = [persist.tile([128, FO], BF16, tag=f"stg{i}",
                                name=f"stg{i}")
                   for i in range(NB)]

            def emit_epilogue(hp_e, accs_e):
                h0e = 2 * hp_e
                for u in range(2):
                    hh = h0e + u
                    rec = epi.tile([128, 8], F32, tag="rec")
                    for half in range(2):
                        nc.vector.reciprocal(
                            rec[:, half * 4:(half + 1) * 4],
                            accs_e[u][half][:, FOH:4 * AUG:AUG],
                        )
                    for ib in range(NB):
                        half, q = divmod(ib, 4)
                        nc.scalar.activation(
                            stg[ib][:, hh * FOH:(hh + 1) * FOH],
                            accs_e[u][half][:, q * AUG:q * AUG + FOH],
                            AF.Copy, scale=rec[:, ib:ib + 1],
                        )
                if hp_e == 1 or hp_e == H // 2 - 1:
                    cse = 0 if hp_e == 1 else FO // 2
                    for ib in range(NB):
                        nc.sync.dma_start(
                            out_d[ib * 128:(ib + 1) * 128, cse:cse + FO // 2],
                            stg[ib][:, cse:cse + FO // 2],
                        )

            # ---- main loop: head pairs, pure-DVE tile path ----
            pending = None
            for hp in range(H // 2):
                h0 = 2 * hp
                accs = [[psAcc.tile([128, 4 * AUG], F32,
                                    tag=f"acc{u}{half}",
                                    name=f"acc{u}{half}",
                                    bufs=(2 if u == 0 else 1))
                         for half in range(2)] for u in range(2)]
                for jb in range(NB):
                    if hp == 0:
                        emit_v12(jb)
                    m2 = mp.tile([128, 2 * N], BF16, tag="m2")
                    for u in range(2):
                        hh = h0 + u
                        nc.vector.tensor_scalar(
                            m2[:, u * N:(u + 1) * N], ub[hh][:],
                            v12[jb][:, hh:hh + 1],
                            v12[jb][:, H + hh:H + hh + 1],
                            OP.mult, OP.max,
                        )
                    rep = (adjt(jb)
                           .rearrange("p (a n) -> p a n", a=1)
                           .to_broadcast([128, 2, N]))
                    nc.vector.tensor_tensor(
                        m2[:].rearrange("p (a n) -> p a n", a=2),
                        m2[:].rearrange("p (a n) -> p a n", a=2),
                        rep, op=OP.mult,
                    )
                    if hp == 0:
                        emit_whaug(jb)
                    if jb == 0 and pending is not None:
                        emit_epilogue(*pending)
                        pending = None
                    for u in range(2):
                        hh = h0 + u
                        for ib in range(NB):
                            half, q = divmod(ib, 4)
                            # 4 accumulation regions share one PSUM bank;
                            # only the bank's first MM may set start
                            # (start clears the whole bank's has_written).
                            nc.tensor.matmul(
                                accs[u][half][:, q * AUG:(q + 1) * AUG],
                                m2[:, u * N + ib * 128:u * N + (ib + 1) * 128],
                                whaug[jb][:, hh * AUG:(hh + 1) * AUG],
                                start=(jb == 0 and q == 0),
                                stop=(jb == NB - 1),
                            )
                pending = (hp, accs)
            emit_epilogue(*pending)

    if split:
        _split_sync_waits(nc)
    return nc


_NC_CACHE = None


def _get_nc():
    global _NC_CACHE
    if _NC_CACHE is None:
        _NC_CACHE = build_nc()
    return _NC_CACHE


def _prep_in_maps(h, adj, W, a):
    h = np.ascontiguousarray(h, dtype=np.float32)
    adj = np.ascontiguousarray(adj, dtype=np.int32)
    W = np.ascontiguousarray(W, dtype=np.float32)
    a = np.ascontiguousarray(a, dtype=np.float32)

    bf = ml_dtypes.bfloat16
    amat = np.zeros((FO, 2 * H), dtype=np.float32)
    for hh in range(H):
        amat[hh * FOH:(hh + 1) * FOH, hh] = a[hh, :FOH]
        amat[hh * FOH:(hh + 1) * FOH, H + hh] = a[hh, FOH:]
    wamat = (W @ amat).astype(np.float32)

    waug = np.zeros((FIN + 1, WAUGC), dtype=np.float32)
    for hh in range(H):
        waug[:FIN, hh * AUG:hh * AUG + FOH] = W[:, hh * FOH:(hh + 1) * FOH]
        waug[FIN, hh * AUG + FOH] = 1.0

    wpack = np.concatenate(
        [waug[0:128], waug[128:256], wamat[0:128], wamat[128:256]], axis=1
    ).astype(bf)
    wones = waug[256:257].astype(bf)

    in_maps = []
    for c in range(N_CORES):
        hT = np.ascontiguousarray(h[c].T)  # [256, 1024]
        hP = hT.reshape(2, 128, N).transpose(1, 0, 2).reshape(128, 2 * N)
        adjT = np.ascontiguousarray(adj[c].T)
        adjP = adjT.reshape(NB, 128, N).transpose(1, 0, 2).reshape(128,
                                                                   NB * N)
        in_maps.append({
            "hP": np.ascontiguousarray(hP).astype(bf),
            "adjP": np.ascontiguousarray(adjP).astype(bf),
            "wpack": wpack,
            "wones": wones,
        })
    return in_maps


def run(h, adj, W, a, trace=False, **kw):
    nc = _get_nc()
    in_maps = _prep_in_maps(h, adj, W, a)
    res = run_bass_kernel_spmd(nc, in_maps, list(range(N_CORES)), trace=trace,
                               **kw)
    out = np.stack([np.asarray(res.results[c]["out"]).astype(np.float32)
                    for c in range(N_CORES)], axis=0)
    return out, res


def kernel(h, adj, W, a):
    out, _ = run(h, adj, W, a)
    return out


# revision 4
# speedup vs baseline: 1.0273x; 1.0273x over previous
"""MultiHeadGAT layer on 8 trn2 NeuronCores, data-parallel over batch — v7.

Math (per core, softmax scale-invariance):
  P'[j,i] = adj[i,j] * max(u[i]*v1[j], v2[j]);  u=exp(.8 a), v1=exp(b),
  v2=exp(.2 b);  a=e_src, b=e_dst.  Exact: the exp(.2 a_i) factor cancels
  in the softmax.

Per [128j, 1024i] tile (pure DVE path; ACT/GPSIMD offloads measured
net-negative: scalar_tensor_tensor runs 1x, GPSIMD contends for the DVE
SBUF port):
  build: fused tensor_scalar  m = (u_b mult v1col) max v2col   (bf16 4x)
  mask:  tensor_tensor paired over 2 heads w/ 0-stride repeat AP on adjT
AV matmul i-on-partitions: lhsT = P-tile [128,128] slices (FWL), rhs =
wh_aug head slice [64 Wh cols | ones] -> denominator in acc col 65k+64.
Epilogue: strided reciprocal + ACT scale-copy into staged bf16 tiles,
half-width output DMAs after hp1/hp3.  Prologue work (whaug, v12) is
interleaved into head-pair 0's jb loop so PE/ACT aren't front-loaded.
Host-side (bf16): hP/adjP pre-transposed+packed, wpack = Waug|WA rows.
"""
import sys

sys.path.insert(0, "/opt/trn_rl_repo")

import numpy as np
import ml_dtypes

import concourse.bass as bass
import concourse.mybir as mybir
import concourse.tile as tile
from concourse.bass_utils import run_bass_kernel_spmd
from concourse.masks import make_identity

F32 = mybir.dt.float32
BF16 = mybir.dt.bfloat16
AF = mybir.ActivationFunctionType
OP = mybir.AluOpType

N_CORES = 8
N = 1024
NB = 8
FIN = 256
H = 8
FOH = 64
FO = H * FOH
AUG = FOH + 1
ALPHA = 0.2
WAUGC = H * AUG  # 520
NSEL = 2         # heads built via PE selector; rest via DMA doubling

_MAX_SYNC_WAITS = 1


def _split_sync_waits(nc, max_waits=_MAX_SYNC_WAITS):
    """This walrus build rejects instructions carrying more than one sync
    wait; hoist extras onto NOPs inserted just before, on the same engine."""
    uid = 0
    for f in nc.m.functions:
        for bb in f.blocks:
            out = []
            for inst in bb.instructions:
                si = getattr(inst, "sync_info", None)
                if si is not None and si.on_wait and len(si.on_wait) > max_waits:
                    waits = list(si.on_wait)
                    keep = waits[-max_waits:]
                    extra = waits[:-max_waits]
                    si.on_wait.clear()
                    si.on_wait.extend(keep)
                    while extra:
                        chunk, extra = extra[:max_waits], extra[max_waits:]
                        nop = mybir.InstNoOp(
                            name=f"waitsplit-{uid}",
                            engine=inst.engine,
                            sync_info=mybir.SyncInfo(
                                on_wait=list(chunk), on_update=[]
                            ),
                            bass_nofuse=True,
                        )
                        uid += 1
                        out.append(nop)
                out.append(inst)
            bb.instructions[:] = out


def build_nc(split=True):
    nc = bass.Bass()
    hp_d = nc.declare_dram_parameter("hP", [128, 2 * N], BF16, isOutput=False)
    adjp_d = nc.declare_dram_parameter("adjP", [128, NB * N], BF16,
                                       isOutput=False)
    wpack_d = nc.declare_dram_parameter("wpack", [128, 2 * WAUGC + 4 * H],
                                        BF16, isOutput=False)
    wones_d = nc.declare_dram_parameter("wones", [1, WAUGC], BF16,
                                        isOutput=False)
    out_d = nc.declare_dram_parameter("out", [128, NB * FO], BF16,
                                      isOutput=True)

    with tile.TileContext(nc) as tc:
        with (
            tc.tile_pool(name="const", bufs=1) as const,
            tc.tile_pool(name="persist", bufs=1) as persist,
            tc.tile_pool(name="mp", bufs=6) as mp,
            tc.tile_pool(name="epi", bufs=2) as epi,
            tc.tile_pool(name="psS", bufs=2, space="PSUM") as psS,
            tc.tile_pool(name="psAcc", bufs=1, space="PSUM") as psAcc,
        ):
            # ---- input DMAs first ----
            wpk = const.tile([128, 2 * WAUGC + 4 * H], BF16, tag="wpk")
            nc.sync.dma_start(wpk[:], wpack_d[:, :])
            wa0 = wpk[:, 0:WAUGC]
            wa1 = wpk[:, WAUGC:2 * WAUGC]
            wak0 = wpk[:, 2 * WAUGC:2 * WAUGC + 2 * H]
            wak1 = wpk[:, 2 * WAUGC + 2 * H:2 * WAUGC + 4 * H]
            wones = const.tile([1, WAUGC], BF16, tag="wones")
            nc.sync.dma_start(wones[:], wones_d[:, :])

            htb = const.tile([128, 2 * N], BF16, tag="htb")
            nc.sync.dma_start(htb[:], hp_d[:, :])
            ht0 = htb[:, 0:N]
            ht1 = htb[:, N:2 * N]

            # adjacency: per-block DMAs so each jb is usable on arrival
            adjb = persist.tile([128, NB * N], BF16, tag="adjb")
            for jb in range(NB):
                nc.sync.dma_start(adjb[:, jb * N:(jb + 1) * N],
                                  adjp_d[:, jb * N:(jb + 1) * N])

            def adjt(jb):
                return adjb[:, jb * N:(jb + 1) * N]

            ident = const.tile([128, 128], F32, tag="ident")
            make_identity(nc, ident[:])
            onesrow = const.tile([1, 128], BF16, tag="onesrow")
            nc.gpsimd.memset(onesrow[:], 1.0)

            # preheat the ACT exp table set while DMAs run
            ph = const.tile([1, 16], F32, tag="ph")
            nc.gpsimd.memset(ph[:], 0.0)
            pho = const.tile([1, 16], F32, tag="pho")
            nc.scalar.activation(pho[:], ph[:], AF.Exp)

            # ---- E_T[16, i]: rows 0..7 e_src, 8..15 e_dst ----
            e_t = const.tile([16, N], F32, tag="eT")
            for c in range(2):
                ps = psS.tile([128, 512], F32, tag="ps")
                nc.tensor.matmul(ps[0:16, :], wak0,
                                 ht0[:, c * 512:(c + 1) * 512],
                                 start=True, stop=False)
                nc.tensor.matmul(ps[0:16, :], wak1,
                                 ht1[:, c * 512:(c + 1) * 512],
                                 start=False, stop=True)
                if c == 0:
                    nc.vector.tensor_copy(e_t[:, c * 512:(c + 1) * 512],
                                          ps[0:16, :])
                else:
                    nc.scalar.copy(e_t[:, c * 512:(c + 1) * 512],
                                   ps[0:16, :])

            # ---- u rows; broadcast: first NSEL heads via PE selector
            # (low latency), rest via DMA log-doubling (no engine cost) ----
            urow = const.tile([H, N], BF16, tag="urow")
            nc.scalar.activation(urow[:], e_t[0:H, :], AF.Exp, scale=0.8)
            ub = [persist.tile([128, N], BF16, tag=f"ub{hh}", name=f"ub{hh}")
                  for hh in range(H)]
            for hh in range(NSEL):
                sel = const.tile([H, 128], BF16, tag=f"sel{hh}",
                                 name=f"sel{hh}")
                nc.gpsimd.memset(sel[:], 0.0)
                nc.gpsimd.affine_select(
                    out=sel[:], in_=sel[:], pattern=[[0, 128]],
                    compare_op=mybir.AluOpType.not_equal, fill=1.0,
                    base=-hh, channel_multiplier=1,
                )
                for c in range(2):
                    ps = psS.tile([128, 512], F32, tag="ps")
                    nc.tensor.matmul(ps[:], sel[:],
                                     urow[:, c * 512:(c + 1) * 512],
                                     start=True, stop=True)
                    if c == 0:
                        nc.vector.tensor_copy(
                            ub[hh][:, c * 512:(c + 1) * 512], ps[:])
                    else:
                        nc.scalar.copy(
                            ub[hh][:, c * 512:(c + 1) * 512], ps[:])
            for hh in range(NSEL, H):
                nc.sync.dma_start(ub[hh][0:1, :], urow[hh:hh + 1, :])
                p = 1
                while p < 128:
                    nc.sync.dma_start(ub[hh][p:2 * p, :], ub[hh][0:p, :])
                    p *= 2

            # ---- deferred prologue pieces, emitted inside hp0's jb loop ----
            v12 = [persist.tile([128, 2 * H], F32, tag=f"v{j}", name=f"v{j}")
                   for j in range(NB)]
            whaug = [persist.tile([128, WAUGC], BF16, tag=f"wha{j}",
                                  name=f"wha{j}")
                     for j in range(NB)]

            def emit_v12(jb):
                tp = psS.tile([128, 512], F32, tag="ps")
                nc.tensor.transpose(tp[:, 0:16],
                                    e_t[:, jb * 128:(jb + 1) * 128],
                                    ident[0:16, 0:16])
                nc.scalar.activation(v12[jb][:, 0:H], tp[:, 8:16], AF.Exp)
                nc.scalar.activation(v12[jb][:, H:2 * H], tp[:, 8:16],
                                     AF.Exp, scale=ALPHA)

            def emit_whaug(jb):
                for half in range(2):
                    cs = half * (WAUGC // 2)
                    ce = (half + 1) * (WAUGC // 2)
                    ps = psS.tile([128, 512], F32, tag="ps")
                    nc.tensor.matmul(ps[:, 0:ce - cs],
                                     ht0[:, jb * 128:(jb + 1) * 128],
                                     wa0[:, cs:ce], start=True, stop=False)
                    nc.tensor.matmul(ps[:, 0:ce - cs],
                                     ht1[:, jb * 128:(jb + 1) * 128],
                                     wa1[:, cs:ce], start=False, stop=False)
                    nc.tensor.matmul(ps[:, 0:ce - cs], onesrow[:],
                                     wones[:, cs:ce], start=False, stop=True)
                    if half == 0:
                        nc.vector.tensor_copy(whaug[jb][:, cs:ce],
                                              ps[:, 0:ce - cs])
                    else:
                        nc.scalar.copy(whaug[jb][:, cs:ce], ps[:, 0:ce - cs])

            # ---- output staging ----
            stgb = persist.tile([128, NB * FO], BF16, tag="stgb")

            def emit_epilogue(hp_e, accs_e):
                h0e = 2 * hp_e
                last = hp_e == H // 2 - 1
                recs = []
                for u in range(2):
                    rec = epi.tile([128, 8], F32, tag="rec")
                    for half in range(2):
                        nc.vector.reciprocal(
                            rec[:, half * 4:(half + 1) * 4],
                            accs_e[u][half][:, FOH:4 * AUG:AUG],
                        )
                    recs.append(rec)
                for ib in range(NB):
                    half, q = divmod(ib, 4)
                    for u in range(2):
                        hh = h0e + u
                        dst = stgb[:, ib * FO + hh * FOH:
                                   ib * FO + (hh + 1) * FOH]
                        srcp = accs_e[u][half][:, q * AUG:q * AUG + FOH]
                        if last and (ib + u) % 2 == 0:
                            nc.vector.tensor_scalar_mul(
                                dst, srcp, recs[u][:, ib:ib + 1])
                        else:
                            nc.scalar.activation(
                                dst, srcp, AF.Copy,
                                scale=recs[u][:, ib:ib + 1],
                            )
                if last:
                    nc.sync.dma_start(out_d[:, :], stgb[:, :])

            # ---- main loop: head pairs, pure-DVE tile path ----
            pending = None
            for hp in range(H // 2):
                h0 = 2 * hp
                accs = [[psAcc.tile([128, 4 * AUG], F32,
                                    tag=f"acc{u}{half}",
                                    name=f"acc{u}{half}",
                                    bufs=(2 if u == 0 else 1))
                         for half in range(2)] for u in range(2)]
                for jb in range(NB):
                    if hp == 0:
                        emit_v12(jb)
                    m2 = mp.tile([128, 2 * N], BF16, tag="m2")
                    for u in range(2):
                        hh = h0 + u
                        nc.vector.tensor_scalar(
                            m2[:, u * N:(u + 1) * N], ub[hh][:],
                            v12[jb][:, hh:hh + 1],
                            v12[jb][:, H + hh:H + hh + 1],
                            OP.mult, OP.max,
                        )
                    rep = (adjt(jb)
                           .rearrange("p (a n) -> p a n", a=1)
                           .to_broadcast([128, 2, N]))
                    nc.vector.tensor_tensor(
                        m2[:].rearrange("p (a n) -> p a n", a=2),
                        m2[:].rearrange("p (a n) -> p a n", a=2),
                        rep, op=OP.mult,
                    )
                    if hp == 0:
                        emit_whaug(jb)
                    if jb == 0 and pending is not None:
                        emit_epilogue(*pending)
                        pending = None
                    for u in range(2):
                        hh = h0 + u
                        for ib in range(NB):
                            half, q = divmod(ib, 4)
                            # 4 accumulation regions share one PSUM bank;
                            # only the bank's first MM may set start
                            # (start clears the whole bank's has_written).
                            nc.tensor.matmul(
                                accs[u][half][:, q * AUG:(q + 1) * AUG],
                                m2[:, u * N + ib * 128:u * N + (ib + 1) * 128],
                                whaug[jb][:, hh * AUG:(hh + 1) * AUG],
                                start=(jb == 0 and q == 0),
                                stop=(jb == NB - 1),
                            )
                pending = (hp, accs)
            emit_epilogue(*pending)

    if split:
        _split_sync_waits(nc)
    return nc


_NC_CACHE = None


def _get_nc():
    global _NC_CACHE
    if _NC_CACHE is None:
        _NC_CACHE = build_nc()
    return _NC_CACHE


def _prep_in_maps(h, adj, W, a):
    h = np.ascontiguousarray(h, dtype=np.float32)
    adj = np.ascontiguousarray(adj, dtype=np.int32)
    W = np.ascontiguousarray(W, dtype=np.float32)
    a = np.ascontiguousarray(a, dtype=np.float32)

    bf = ml_dtypes.bfloat16
    amat = np.zeros((FO, 2 * H), dtype=np.float32)
    for hh in range(H):
        amat[hh * FOH:(hh + 1) * FOH, hh] = a[hh, :FOH]
        amat[hh * FOH:(hh + 1) * FOH, H + hh] = a[hh, FOH:]
    wamat = (W @ amat).astype(np.float32)

    waug = np.zeros((FIN + 1, WAUGC), dtype=np.float32)
    for hh in range(H):
        waug[:FIN, hh * AUG:hh * AUG + FOH] = W[:, hh * FOH:(hh + 1) * FOH]
        waug[FIN, hh * AUG + FOH] = 1.0

    wpack = np.concatenate(
        [waug[0:128], waug[128:256], wamat[0:128], wamat[128:256]], axis=1
    ).astype(bf)
    wones = waug[256:257].astype(bf)

    in_maps = []
    for c in range(N_CORES):
        hT = np.ascontiguousarray(h[c].T)  # [256, 1024]
        hP = hT.reshape(2, 128, N).transpose(1, 0, 2).reshape(128, 2 * N)
        adjT = np.ascontiguousarray(adj[c].T)
        adjP = adjT.reshape(NB, 128, N).transpose(1, 0, 2).reshape(128,
                                                                   NB * N)
        in_maps.append({
            "hP": np.ascontiguousarray(hP).astype(bf),
            "adjP": np.ascontiguousarray(adjP).astype(bf),
            "wpack": wpack,
            "wones": wones,
        })
    return in_maps


def run(h, adj, W, a, trace=False, **kw):
    nc = _get_nc()
    in_maps = _prep_in_maps(h, adj, W, a)
    res = run_bass_kernel_spmd(nc, in_maps, list(range(N_CORES)), trace=trace,
                               **kw)
    outs = []
    for c in range(N_CORES):
        a = np.asarray(res.results[c]["out"]).astype(np.float32)
        outs.append(a.reshape(128, NB, FO).transpose(1, 0, 2)
                    .reshape(N, FO))
    return np.stack(outs, axis=0), res


def kernel(h, adj, W, a):
    out, _ = run(h, adj, W, a)
    return out
